# revision 44
# baseline (speedup 1.0000x reference)
"""AdaptiveMultiWIRE on 8 TRN2 NeuronCores — point-major rewrite.

Sharding: C=16 channels over 8 cores (2 channels/core), zero collectives.
All index gathers happen host-side in numpy.

Core idea vs the feature-major v1: activations are the matmul's
STATIONARY operand (lhsT, feature-major K-chunks 128/128/107) and the
packed weights STREAM as rhs [K, 724], so each psum wave is
[128 points, 724 group-columns].  Consequences:

  - PE: 3 K-chunks instead of 4 (363 rows pack into 3 partition blocks
    with zero M-waste: points are 128-aligned).  ~0.94 K-efficiency vs
    ~0.50 for the old {128,53}x{128,53} tiling.
  - every elementwise op runs on [128, free] slices with ZERO partition
    waste (the old layout paid 2x: a 53-partition instr costs the same
    as a 128-partition one).  Group alignment is free-dim slicing.
  - the activation outputs are point-major; the combine phase writes
    them straight into a 3-K-section staging layout and 12 quarter-width
    DMA xbar-transposes (dma_start_transpose, blockwise [f,(b,r)]
    semantics) rebuild the feature-major lhsT tiles on the DMA queues --
    no PE transposes, no fp16 psum, no eviction copies.

Math per layer (same fused Gabor chain as v1):
  g0 = (OMEGA/2pi)*(la.re + b)            phase in "turns"
  g1 = S*(la.im + b) + OMEGA/2S           complete-the-square form
  g2 = S*(lb.re + b);  g3 = S*(lb.im + b)
  g4 = S0*g0 = S*(la.re + b)              extra weight columns so ONE
       Square pass covers all four |.|^2 terms (no separate u0 op)
  f  = g0 - round(g0)  (fp32 magic-number round on DVE)
  E  = exp(-(g1^2 + g2^2 + g3^2 + g4^2))  (carries e^-EBIAS)
  s = Sin(2pi f),  sh = Sin(pi f)  (HW Sin is only valid on [-pi, pi])
  xre' = (sh^2 - 1/2) * E  = -cos(2pi g0)*E/2   (half-angle cosine; the
         -2x is folded into the next layer's re-input weight rows)
  xim' = E * s

Scheduling: engines execute in issue order.  The two channels run a
half-layer apart (late(ch); early(ch, next layer) per channel), and the
finals prime the next batch's layer-0 so PE never drains at batch
boundaries.  Sins issue inline mid-early in halves (Square lives in
every activation table -> no extra loads; only Sin<->Exp transitions
pay 1283ns, 2 loads/layer).  Squares: ScalarE is the only engine that
can square psum (one-psum-operand rule), so 3/8 of each layer's tiles
are offloaded as DVE-evict + Pool-square, with v-adds and the wv sum
accumulated per-tile on Pool during the matmul stream.  The psum pool
rotates 2x [128,2048]f32 4-bank buffers shared (by tag) with the
final-layer psf.
"""

import numpy as np

C, N, H, OUT, NIN, NSRC, NB = 16, 8192, 181, 3, 2, 32, 8
OMEGA, SCALE = 30.0, 10.0
NCORES, CPC = 8, 2
PI = float(np.pi)
R2 = OMEGA / (2.0 * PI)          # turns per unit la.re
S0 = SCALE / R2
EBIAS = OMEGA * OMEGA / (4.0 * SCALE * SCALE)   # 2.25
MAGIC = 12582912.0               # 1.5*2^23: fp32 add/sub rounds to int
PB = 2048                        # points per batch
NBATCH = N // PB                 # 4
PC = 128                         # points per chunk (psum partition dim)
NPC = PB // PC                   # 16 chunks per batch
NW = 5 * H                       # 905 hidden wave columns (g0..g3 + g4=S0*g0)
NW0 = 3 * H                      # 543 layer-0 wave columns (g0, g2', g4')
KC = [(0, 128), (128, 256), (256, 363)]   # K row chunks of 2H+1

_GRAPH = None


def _build_graph():
    import concourse.mybir as mybir
    from concourse import bacc
    from concourse.tile import TileContext

    dt = mybir.dt
    f16, f32 = dt.float16, dt.float32
    Alu = mybir.AluOpType
    Act = mybir.ActivationFunctionType

    nc = bacc.Bacc()
    xa_d = nc.declare_dram_parameter("xa", [CPC, 3, N], f16, isOutput=False)
    w0_d = nc.declare_dram_parameter("w0", [CPC, 3, NW0], f16, isOutput=False)
    w1_d = nc.declare_dram_parameter("w1", [CPC, 2 * H + 1, NW], f16,
                                     isOutput=False)
    w2_d = nc.declare_dram_parameter("w2", [CPC, 2 * H + 1, NW], f16,
                                     isOutput=False)
    wf_d = nc.declare_dram_parameter("wf", [CPC, 2 * H + 1, OUT], f16,
                                     isOutput=False)
    idn_d = nc.declare_dram_parameter("idn", [128, 128], f16, isOutput=False)
    out_d = nc.declare_dram_parameter("out", [CPC, OUT, N], f16, isOutput=True)

    with TileContext(nc) as tc:
        with (
            tc.tile_pool(name="wpool", bufs=1) as wpool,
            tc.tile_pool(name="xpool", bufs=1) as xpool,
            tc.tile_pool(name="apool", bufs=2) as apool,
            tc.tile_pool(name="spool", bufs=1) as spool,
            tc.tile_pool(name="kpool", bufs=2) as kpool,
            tc.tile_pool(name="psum", bufs=2, space="PSUM") as pp,
        ):
            # ---- persistent loads ------------------------------------
            idn = wpool.tile([128, 128], f16, tag="idn", name="idn")
            nc.sync.dma_start(out=idn[:], in_=idn_d[:])
            w0t, wts, wfts = [], [], []
            for ch in range(CPC):
                t = wpool.tile([3, NW0], f16, tag=f"w0{ch}", name=f"w0{ch}")
                nc.sync.dma_start(out=t[:], in_=w0_d[ch])
                w0t.append(t)
                per_layer = []
                for li, wd in ((1, w1_d), (2, w2_d)):
                    tiles = []
                    for ki, (r0, r1) in enumerate(KC):
                        t = wpool.tile([r1 - r0, NW], f16, tag=f"w{li}{ch}k{ki}")
                        nc.sync.dma_start(out=t[:], in_=wd[ch, r0:r1, :])
                        tiles.append(t)
                    per_layer.append(tiles)
                wts.append(per_layer)
                tiles = []
                for ki, (r0, r1) in enumerate(KC):
                    t = wpool.tile([r1 - r0, OUT], f16, tag=f"wf{ch}k{ki}")
                    nc.sync.dma_start(out=t[:], in_=wf_d[ch, r0:r1, :])
                    tiles.append(t)
                wfts.append(tiles)

            def expph(ch, wv):
                W = NPC * H
                E = spool.tile([128, W], f16, tag=f"E{ch}")
                nc.scalar.activation(E[:], wv[:], Act.Exp, bias=0.0,
                                     scale=-1.0)
                return E

            def mm_early(ch, li, xan, X):
                """Matmuls + psum-bound early ops + per-tile v-adds +
                inline half-batch Sins + wide wv + half-batch Exps.
                Returns (s, sh, E) wide tiles."""
                gw = 2 * H if li == 0 else 4 * H   # square-section width
                W = NPC * H
                HW2 = W // 2
                fw = spool.tile([128, W], f16, tag=f"fw{ch}")
                sqw = spool.tile([128, NPC * 4 * H], f16, tag=f"sqw{ch}")
                s = spool.tile([128, W], f16, tag=f"s{ch}")
                sh = spool.tile([128, W], f16, tag=f"sh{ch}")
                wvw = spool.tile([128, W], f16, tag=f"wv{ch}")
                if li > 0:
                    v1w = spool.tile([128, W], f16, tag=f"v1{ch}")
                    v2w = spool.tile([128, W], f16, tag=f"v2{ch}")
                for t in range(NPC // 2):
                    ps = pp.tile([128, 2048], f32, tag="wav")
                    for slot in (0, 1):
                        pc = 2 * t + slot
                        o = slot * 1024
                        if li == 0:
                            lhs = xan[:, pc * PC:(pc + 1) * PC]
                            nc.tensor.matmul(ps[:, o:o + 512], lhsT=lhs,
                                             rhs=w0t[ch][:, 0:512],
                                             start=True, stop=True)
                            nc.tensor.matmul(ps[:, o + 512:o + NW0], lhsT=lhs,
                                             rhs=w0t[ch][:, 512:NW0],
                                             start=True, stop=True)
                        else:
                            wk = wts[ch][li - 1]
                            T0, T1, T2 = X
                            for ki, xt in enumerate((T0, T1, T2)):
                                kr = KC[ki][1] - KC[ki][0]
                                lhs = xt[0:kr, pc * PC:(pc + 1) * PC]
                                nc.tensor.matmul(ps[:, o:o + 512],
                                                 lhsT=lhs, rhs=wk[ki][:, 0:512],
                                                 start=(ki == 0), stop=(ki == 2))
                                nc.tensor.matmul(ps[:, o + 512:o + NW],
                                                 lhsT=lhs, rhs=wk[ki][:, 512:NW],
                                                 start=(ki == 0), stop=(ki == 2))
                    ps3 = ps[:].rearrange("p (s w) -> p s w", w=1024)
                    g0 = ps3[:, :, 0:H]
                    k1 = kpool.tile([128, 2 * H], f32, tag=f"k1{ch}")
                    nc.vector.tensor_scalar(k1[:], g0, MAGIC, MAGIC,
                                            Alu.add, Alu.subtract)
                    nc.vector.scalar_tensor_tensor(
                        fw[:, t * 2 * H:(t + 1) * 2 * H], k1[:], -1.0, g0,
                        Alu.mult, Alu.add)
                    if t in ((1, 4, 6) if li > 0 else (1, 3, 5)):
                        # offload this tile's squares: DVE evicts the psum
                        # sections, Pool squares them (ScalarE is the
                        # bottleneck and the only engine that can square
                        # psum directly).  ev reuses the E buffer, which
                        # is dead during the early phase.
                        ev = xpool.tile([128, 2 * gw], f16, tag=f"ev{ch}",
                                        name=f"ev{ch}")
                        nc.vector.tensor_copy(ev[:], ps3[:, :, H:H + gw])
                        nc.gpsimd.tensor_tensor(
                            sqw[:, t * 2 * gw:(t + 1) * 2 * gw],
                            ev[:], ev[:], Alu.mult)
                    else:
                        nc.scalar.activation(
                            sqw[:, t * 2 * gw:(t + 1) * 2 * gw],
                            ps3[:, :, H:H + gw], Act.Square, bias=0.0,
                            scale=1.0)
                    sq3t = sqw[:, t * 2 * gw:(t + 1) * 2 * gw].rearrange(
                        "p (s g) -> p s g", g=gw)
                    tsl = slice(t * 2 * H, (t + 1) * 2 * H)
                    if li > 0:
                        # per-tile v-adds and wv over this tile's sq
                        # sections so they overlap the remaining matmuls
                        nc.gpsimd.tensor_tensor(v1w[:, tsl], sq3t[:, :, 0:H],
                                                sq3t[:, :, H:2 * H], Alu.add)
                        nc.gpsimd.tensor_tensor(v2w[:, tsl],
                                                sq3t[:, :, 2 * H:3 * H],
                                                sq3t[:, :, 3 * H:4 * H],
                                                Alu.add)
                        nc.gpsimd.tensor_tensor(wvw[:, tsl], v1w[:, tsl],
                                                v2w[:, tsl], Alu.add)
                    else:
                        nc.gpsimd.tensor_tensor(wvw[:, tsl], sq3t[:, :, 0:H],
                                                sq3t[:, :, H:2 * H], Alu.add)
                    if t == NPC // 4 - 1:
                        # first-half Sins fire while the second half's
                        # matmuls stream
                        nc.scalar.activation(s[:, 0:HW2], fw[:, 0:HW2],
                                             Act.Sin, bias=0.0, scale=2 * PI)
                        nc.scalar.activation(sh[:, 0:HW2], fw[:, 0:HW2],
                                             Act.Sin, bias=0.0, scale=PI)
                nc.scalar.activation(s[:, HW2:W], fw[:, HW2:W], Act.Sin,
                                     bias=0.0, scale=2 * PI)
                nc.scalar.activation(sh[:, HW2:W], fw[:, HW2:W], Act.Sin,
                                     bias=0.0, scale=PI)
                return s, sh, wvw

            def combine_transpose(ch, s, sh, E):
                """c2/xre/xim written straight into the 3 K-section layout
                xq (each section = 16 point-blocks x 128 rows), then 12
                quarter-width DMA xbar-transposes build the next layer's
                feature-major T tiles -- no PE transposes, no psum."""
                W = NPC * H
                c2 = spool.tile([128, W], f16, tag=f"v1{ch}")
                nc.gpsimd.tensor_tensor(c2[:], sh[:], sh[:], Alu.mult)
                # xq aliases the sqw buffer (dead after the v-adds)
                xq = spool.tile([128, 3 * PB], f16, tag=f"sqw{ch}")
                q0 = xq[:, 0:PB].rearrange("p (n r) -> p n r", r=128)
                q1 = xq[:, PB:2 * PB].rearrange("p (n r) -> p n r", r=128)
                q2 = xq[:, 2 * PB:3 * PB].rearrange("p (n r) -> p n r", r=128)
                c3 = c2[:].rearrange("p (n g) -> p n g", g=H)
                E3 = E[:].rearrange("p (n g) -> p n g", g=H)
                s3 = s[:].rearrange("p (n g) -> p n g", g=H)
                # xre rows 0:128 -> sec0; rows 128:181 -> sec1[0:53]
                nc.vector.scalar_tensor_tensor(
                    q0[:, :, :], c3[:, :, 0:128], 0.5, E3[:, :, 0:128],
                    Alu.subtract, Alu.mult)
                nc.vector.scalar_tensor_tensor(
                    q1[:, :, 0:53], c3[:, :, 128:H], 0.5, E3[:, :, 128:H],
                    Alu.subtract, Alu.mult)
                # xim rows 0:75 -> sec1[53:128]; rows 75:181 -> sec2[0:106]
                nc.gpsimd.tensor_tensor(q1[:, :, 53:128], E3[:, :, 0:75],
                                        s3[:, :, 0:75], Alu.mult)
                nc.gpsimd.tensor_tensor(q2[:, :, 0:106], E3[:, :, 75:H],
                                        s3[:, :, 75:H], Alu.mult)
                # ones row (106) + harmless padding rows 107..127
                nc.vector.memset(q2[:, :, 106:128], 1.0)
                T0 = xpool.tile([128, PB], f16, tag=f"T0{ch}", name=f"T0{ch}")
                T1 = xpool.tile([128, PB], f16, tag=f"T1{ch}", name=f"T1{ch}")
                T2 = xpool.tile([128, PB], f16, tag=f"T2{ch}", name=f"T2{ch}")
                for part in range(4):
                    psl = slice(part * 512, (part + 1) * 512)
                    for sec, Tt in enumerate((T0, T1, T2)):
                        t3 = Tt[:, psl].rearrange("f (b r) -> f b r", r=128)
                        nc.sync.dma_start_transpose(
                            t3, xq[:, sec * PB + part * 512:
                                   sec * PB + (part + 1) * 512])
                return T0, T1, T2

            # ---- main loop -------------------------------------------
            def load_xa(nb):
                d = {}
                for ch in range(CPC):
                    t = apool.tile([3, PB], f16, tag=f"xa{ch}")
                    nc.sync.dma_start(out=t[:],
                                      in_=xa_d[ch, :, nb * PB:(nb + 1) * PB])
                    d[ch] = t
                return d

            X = {ch: None for ch in range(CPC)}

            def late(ch, mid, Ee):
                s, sh, wv = mid
                X[ch] = combine_transpose(ch, s, sh, Ee)

            def final(ch, nb):
                T0, T1, T2 = X[ch]
                psf = pp.tile([OUT, PB], f32, tag="wav")
                for ni in range(PB // 512):
                    sl = slice(ni * 512, (ni + 1) * 512)
                    for ki, xt in enumerate((T0, T1, T2)):
                        kr = KC[ki][1] - KC[ki][0]
                        nc.tensor.matmul(psf[:, sl], lhsT=wfts[ch][ki][:],
                                         rhs=xt[0:kr, sl],
                                         start=(ki == 0), stop=(ki == 2))
                ob = spool.tile([OUT, PB], f16, tag=f"ob{ch}")
                nc.vector.tensor_copy(ob[:], psf[:])
                nc.sync.dma_start(out=out_d[ch, :, nb * PB:(nb + 1) * PB],
                                  in_=ob[:])

            # phase order: early both channels, then late both
            xan = load_xa(0)
            for nb in range(NBATCH):
                if nb == 0:
                    mid = {}
                    mid[0] = mm_early(0, 0, xan[0], X[0])
                    mid[1] = mm_early(1, 0, xan[1], X[1])
                xan_next = load_xa(nb + 1) if nb + 1 < NBATCH else None
                for li in (0, 1, 2):
                    # both channels' Exps issue back-to-back (one table
                    # load instead of two: the pipeline otherwise lands
                    # each Exp between the other channel's Sin blocks)
                    Ee = {ch: expph(ch, mid[ch][2]) for ch in range(CPC)}
                    for ch in range(CPC):
                        late(ch, mid[ch], Ee[ch])
                        if li < 2:
                            mid[ch] = mm_early(ch, li + 1, xan[ch], X[ch])
                        else:
                            final(ch, nb)
                            if xan_next is not None:
                                mid[ch] = mm_early(ch, 0, xan_next[ch],
                                                   X[ch])
                xan = xan_next
    nc.finalize()
    return nc


def _get_graph():
    global _GRAPH
    if _GRAPH is None:
        _GRAPH = _build_graph()
    return _GRAPH


def _pack_inputs(inp, indices, model_idx, bias_idx, W0a, b0a, W0b, b0b,
                 W1a, b1a, W1b, b1b, W2a, b2a, W2b, b2b, Wf, bf):
    """Host-side gather + weight packing. Returns in_maps for 8 cores.

    Weight rows are input features in T-tile order [xre'(181); xim'(181);
    ones]; columns are the 4 group outputs [g0|g1|g2|g3].  alpha/beta are
    the affine factors mapping stored activations to true ones:
    x.re = alpha*xre', x.im = beta*xim'.
    """
    cplx = lambda a: a[..., 0] + 1j * a[..., 1]
    idn = np.eye(128, dtype=np.float16)

    def pack_hidden(Wa, Wb, ba, bb, alpha, beta):
        # columns [g0 | g1 | g2 | g3 | g4] with g4 = S0*g0 (so the
        # sum-of-squares pass covers (S*la.re)^2 too, no separate u0)
        re_rows = np.concatenate([
            R2 * alpha * Wa.real, SCALE * alpha * Wa.imag,
            SCALE * alpha * Wb.real, SCALE * alpha * Wb.imag,
            SCALE * alpha * Wa.real], axis=1)
        im_rows = np.concatenate([
            -R2 * beta * Wa.imag, SCALE * beta * Wa.real,
            -SCALE * beta * Wb.imag, SCALE * beta * Wb.real,
            -SCALE * beta * Wa.imag], axis=1)
        ones_row = np.concatenate([
            R2 * ba.real, SCALE * ba.imag + OMEGA / (2 * SCALE),
            SCALE * bb.real, SCALE * bb.imag, SCALE * ba.real])[None, :]
        return np.concatenate([re_rows, im_rows, ones_row],
                              axis=0).astype(np.float16)

    a0, b0c = -2.0, 1.0                          # L0 -> L1 factors
    a1, b1c = -2.0 * np.exp(EBIAS), np.exp(EBIAS)  # L1 -> L2, L2 -> final

    in_maps = []
    for core in range(NCORES):
        m = {k: [] for k in ("xa", "w0", "w1", "w2", "wf")}
        for j in range(CPC):
            c = core * CPC + j
            mi, bi = int(model_idx[c]), int(bias_idx[c])
            x = inp[int(indices[c])]          # [N, NIN]
            m["xa"].append(np.concatenate(
                [x.T, np.ones((1, N), np.float32)], 0).astype(np.float16))
            w0blk = np.concatenate([
                np.concatenate([R2 * W0a[mi], SCALE * W0b[mi],
                                SCALE * W0a[mi]], axis=1),
                np.concatenate([R2 * b0a[bi], SCALE * b0b[bi],
                                SCALE * b0a[bi]])[None, :],
            ], axis=0)
            m["w0"].append(w0blk.astype(np.float16))
            m["w1"].append(pack_hidden(cplx(W1a[mi]), cplx(W1b[mi]),
                                       cplx(b1a[bi]), cplx(b1b[bi]), a0, b0c))
            m["w2"].append(pack_hidden(cplx(W2a[mi]), cplx(W2b[mi]),
                                       cplx(b2a[bi]), cplx(b2b[bi]), a1, b1c))
            Wfc, bfc = cplx(Wf[mi]), cplx(bf[bi])
            wfblk = np.concatenate([
                a1 * Wfc.real, -b1c * Wfc.imag, bfc.real[None, :]],
                axis=0).astype(np.float16)
            m["wf"].append(wfblk)
        packed = {k: np.stack(v) for k, v in m.items()}
        packed["idn"] = idn
        in_maps.append(packed)
    return in_maps


def kernel(**inputs):
    inp = np.asarray(inputs["inp"], np.float32)
    args = {k: np.asarray(v) for k, v in inputs.items()}
    in_maps = _pack_inputs(
        inp, args["indices"], args["model_idx"], args["bias_idx"],
        *[np.asarray(args[k], np.float32) for k in
          ("W0a", "b0a", "W0b", "b0b", "W1a", "b1a", "W1b", "b1b",
           "W2a", "b2a", "W2b", "b2b", "Wf", "bf")])
    from concourse.bass_utils import run_bass_kernel_spmd
    nc = _get_graph()
    res = run_bass_kernel_spmd(nc, in_maps, core_ids=list(range(NCORES)))
    out = np.empty((1, C, N, OUT), np.float32)
    for core in range(NCORES):
        o = np.asarray(res.results[core]["out"])   # [CPC, OUT, N] fp16
        for j in range(CPC):
            out[0, core * CPC + j] = o[j].T.astype(np.float32)
    return out


if __name__ == "__main__":
    import jax
    import reference
    cpu = jax.devices("cpu")[0]
    with jax.default_device(cpu):
        ins = {k: np.asarray(v) for k, v in reference.setup_inputs().items()}
        exp = np.asarray(reference.reference(
            **{k: jax.device_put(v, cpu) for k, v in ins.items()}))
    got = kernel(**ins)
    rel = np.linalg.norm(got - exp) / np.linalg.norm(exp)
    print("Relative error:", rel)


# revision 45
# speedup vs baseline: 1.0101x; 1.0101x over previous
"""AdaptiveMultiWIRE on 8 TRN2 NeuronCores — point-major rewrite.

Sharding: C=16 channels over 8 cores (2 channels/core), zero collectives.
All index gathers happen host-side in numpy.

Core idea vs the feature-major v1: activations are the matmul's
STATIONARY operand (lhsT, feature-major K-chunks 128/128/107) and the
packed weights STREAM as rhs [K, 724], so each psum wave is
[128 points, 724 group-columns].  Consequences:

  - PE: 3 K-chunks instead of 4 (363 rows pack into 3 partition blocks
    with zero M-waste: points are 128-aligned).  ~0.94 K-efficiency vs
    ~0.50 for the old {128,53}x{128,53} tiling.
  - every elementwise op runs on [128, free] slices with ZERO partition
    waste (the old layout paid 2x: a 53-partition instr costs the same
    as a 128-partition one).  Group alignment is free-dim slicing.
  - the activation outputs are point-major; the combine phase writes
    them straight into a 3-K-section staging layout and 12 quarter-width
    DMA xbar-transposes (dma_start_transpose, blockwise [f,(b,r)]
    semantics) rebuild the feature-major lhsT tiles on the DMA queues --
    no PE transposes, no fp16 psum, no eviction copies.

Math per layer (same fused Gabor chain as v1):
  g0 = (OMEGA/2pi)*(la.re + b)            phase in "turns"
  g1 = S*(la.im + b) + OMEGA/2S           complete-the-square form
  g2 = S*(lb.re + b);  g3 = S*(lb.im + b)
  g4 = S0*g0 = S*(la.re + b)              extra weight columns so ONE
       Square pass covers all four |.|^2 terms (no separate u0 op)
  f  = g0 - round(g0)  (fp32 magic-number round on DVE)
  E  = exp(-(g1^2 + g2^2 + g3^2 + g4^2))  (carries e^-EBIAS)
  s = Sin(2pi f),  sh = Sin(pi f)  (HW Sin is only valid on [-pi, pi])
  xre' = (sh^2 - 1/2) * E  = -cos(2pi g0)*E/2   (half-angle cosine; the
         -2x is folded into the next layer's re-input weight rows)
  xim' = E * s

Scheduling: engines execute in issue order.  The two channels run a
half-layer apart (late(ch); early(ch, next layer) per channel), and the
finals prime the next batch's layer-0 so PE never drains at batch
boundaries.  Sins issue inline mid-early in halves (Square lives in
every activation table -> no extra loads; only Sin<->Exp transitions
pay 1283ns, 2 loads/layer).  Squares: ScalarE is the only engine that
can square psum (one-psum-operand rule), so 3/8 of each layer's tiles
are offloaded as DVE-evict + Pool-square, with v-adds and the wv sum
accumulated per-tile on Pool during the matmul stream.  The psum pool
rotates 2x [128,2048]f32 4-bank buffers shared (by tag) with the
final-layer psf.
"""

import numpy as np

C, N, H, OUT, NIN, NSRC, NB = 16, 8192, 181, 3, 2, 32, 8
OMEGA, SCALE = 30.0, 10.0
NCORES, CPC = 8, 2
PI = float(np.pi)
R2 = OMEGA / (2.0 * PI)          # turns per unit la.re
S0 = SCALE / R2
EBIAS = OMEGA * OMEGA / (4.0 * SCALE * SCALE)   # 2.25
MAGIC = 12582912.0               # 1.5*2^23: fp32 add/sub rounds to int
PB = 2048                        # points per batch
NBATCH = N // PB                 # 4
PC = 128                         # points per chunk (psum partition dim)
NPC = PB // PC                   # 16 chunks per batch
NW = 5 * H                       # 905 hidden wave columns (g0..g3 + g4=S0*g0)
NW0 = 3 * H                      # 543 layer-0 wave columns (g0, g2', g4')
KC = [(0, 128), (128, 256), (256, 363)]   # K row chunks of 2H+1

_GRAPH = None


def _build_graph():
    import concourse.mybir as mybir
    from concourse import bacc
    from concourse.tile import TileContext

    dt = mybir.dt
    f16, f32 = dt.float16, dt.float32
    Alu = mybir.AluOpType
    Act = mybir.ActivationFunctionType

    nc = bacc.Bacc()
    xa_d = nc.declare_dram_parameter("xa", [CPC, 3, N], f16, isOutput=False)
    w0_d = nc.declare_dram_parameter("w0", [CPC, 3, NW0], f16, isOutput=False)
    w1_d = nc.declare_dram_parameter("w1", [CPC, 2 * H + 1, NW], f16,
                                     isOutput=False)
    w2_d = nc.declare_dram_parameter("w2", [CPC, 2 * H + 1, NW], f16,
                                     isOutput=False)
    wf_d = nc.declare_dram_parameter("wf", [CPC, 2 * H + 1, OUT], f16,
                                     isOutput=False)
    idn_d = nc.declare_dram_parameter("idn", [128, 128], f16, isOutput=False)
    out_d = nc.declare_dram_parameter("out", [CPC, OUT, N], f16, isOutput=True)

    with TileContext(nc) as tc:
        with (
            tc.tile_pool(name="wpool", bufs=1) as wpool,
            tc.tile_pool(name="xpool", bufs=1) as xpool,
            tc.tile_pool(name="apool", bufs=2) as apool,
            tc.tile_pool(name="spool", bufs=1) as spool,
            tc.tile_pool(name="kpool", bufs=2) as kpool,
            tc.tile_pool(name="psum", bufs=2, space="PSUM") as pp,
        ):
            # ---- persistent loads ------------------------------------
            idn = wpool.tile([128, 128], f16, tag="idn", name="idn")
            nc.sync.dma_start(out=idn[:], in_=idn_d[:])
            w0t, wts, wfts = [], [], []
            for ch in range(CPC):
                t = wpool.tile([3, NW0], f16, tag=f"w0{ch}", name=f"w0{ch}")
                nc.sync.dma_start(out=t[:], in_=w0_d[ch])
                w0t.append(t)
                per_layer = []
                for li, wd in ((1, w1_d), (2, w2_d)):
                    tiles = []
                    for ki, (r0, r1) in enumerate(KC):
                        t = wpool.tile([r1 - r0, NW], f16, tag=f"w{li}{ch}k{ki}")
                        nc.sync.dma_start(out=t[:], in_=wd[ch, r0:r1, :])
                        tiles.append(t)
                    per_layer.append(tiles)
                wts.append(per_layer)
                tiles = []
                for ki, (r0, r1) in enumerate(KC):
                    t = wpool.tile([r1 - r0, OUT], f16, tag=f"wf{ch}k{ki}")
                    nc.sync.dma_start(out=t[:], in_=wf_d[ch, r0:r1, :])
                    tiles.append(t)
                wfts.append(tiles)

            def expph(ch, wv):
                W = NPC * H
                E = spool.tile([128, W], f16, tag=f"E{ch}")
                nc.scalar.activation(E[:], wv[:], Act.Exp, bias=0.0,
                                     scale=-1.0)
                return E

            def mm_early(ch, li, xan, X):
                """Matmuls + psum-bound early ops + per-tile v-adds +
                inline half-batch Sins + wide wv + half-batch Exps.
                Returns (s, sh, E) wide tiles."""
                gw = 2 * H if li == 0 else 4 * H   # square-section width
                W = NPC * H
                HW2 = W // 2
                fw = spool.tile([128, W], f16, tag=f"fw{ch}")
                sqw = spool.tile([128, NPC * 4 * H], f16, tag=f"sqw{ch}")
                s = spool.tile([128, W], f16, tag=f"s{ch}")
                sh = spool.tile([128, W], f16, tag=f"sh{ch}")
                wvw = spool.tile([128, W], f16, tag=f"wv{ch}")
                if li > 0:
                    v1w = spool.tile([128, W], f16, tag=f"v1{ch}")
                    v2w = spool.tile([128, W], f16, tag=f"v2{ch}")
                for t in range(NPC // 2):
                    ps = pp.tile([128, 2048], f32, tag="wav")
                    for slot in (0, 1):
                        pc = 2 * t + slot
                        o = slot * 1024
                        if li == 0:
                            lhs = xan[:, pc * PC:(pc + 1) * PC]
                            nc.tensor.matmul(ps[:, o:o + 512], lhsT=lhs,
                                             rhs=w0t[ch][:, 0:512],
                                             start=True, stop=True)
                            nc.tensor.matmul(ps[:, o + 512:o + NW0], lhsT=lhs,
                                             rhs=w0t[ch][:, 512:NW0],
                                             start=True, stop=True)
                        else:
                            wk = wts[ch][li - 1]
                            T0, T1, T2 = X
                            for ki, xt in enumerate((T0, T1, T2)):
                                kr = KC[ki][1] - KC[ki][0]
                                lhs = xt[0:kr, pc * PC:(pc + 1) * PC]
                                nc.tensor.matmul(ps[:, o:o + 512],
                                                 lhsT=lhs, rhs=wk[ki][:, 0:512],
                                                 start=(ki == 0), stop=(ki == 2))
                                nc.tensor.matmul(ps[:, o + 512:o + NW],
                                                 lhsT=lhs, rhs=wk[ki][:, 512:NW],
                                                 start=(ki == 0), stop=(ki == 2))
                    ps3 = ps[:].rearrange("p (s w) -> p s w", w=1024)
                    g0 = ps3[:, :, 0:H]
                    k1 = kpool.tile([128, 2 * H], f32, tag=f"k1{ch}")
                    nc.vector.tensor_scalar(k1[:], g0, MAGIC, MAGIC,
                                            Alu.add, Alu.subtract)
                    nc.vector.scalar_tensor_tensor(
                        fw[:, t * 2 * H:(t + 1) * 2 * H], k1[:], -1.0, g0,
                        Alu.mult, Alu.add)
                    if t in ((1, 4, 6) if li > 0 else (1, 3, 5)):
                        # offload this tile's squares: DVE evicts the psum
                        # sections, Pool squares them (ScalarE is the
                        # bottleneck and the only engine that can square
                        # psum directly).  ev reuses the E buffer, which
                        # is dead during the early phase.
                        ev = xpool.tile([128, 2 * gw], f16, tag=f"ev{ch}",
                                        name=f"ev{ch}")
                        nc.vector.tensor_copy(ev[:], ps3[:, :, H:H + gw])
                        nc.gpsimd.tensor_tensor(
                            sqw[:, t * 2 * gw:(t + 1) * 2 * gw],
                            ev[:], ev[:], Alu.mult)
                    else:
                        nc.scalar.activation(
                            sqw[:, t * 2 * gw:(t + 1) * 2 * gw],
                            ps3[:, :, H:H + gw], Act.Square, bias=0.0,
                            scale=1.0)
                    sq3t = sqw[:, t * 2 * gw:(t + 1) * 2 * gw].rearrange(
                        "p (s g) -> p s g", g=gw)
                    tsl = slice(t * 2 * H, (t + 1) * 2 * H)
                    if li > 0:
                        # per-tile v-adds and wv over this tile's sq
                        # sections so they overlap the remaining matmuls
                        nc.gpsimd.tensor_tensor(v1w[:, tsl], sq3t[:, :, 0:H],
                                                sq3t[:, :, H:2 * H], Alu.add)
                        nc.gpsimd.tensor_tensor(v2w[:, tsl],
                                                sq3t[:, :, 2 * H:3 * H],
                                                sq3t[:, :, 3 * H:4 * H],
                                                Alu.add)
                        nc.gpsimd.tensor_tensor(wvw[:, tsl], v1w[:, tsl],
                                                v2w[:, tsl], Alu.add)
                    else:
                        nc.gpsimd.tensor_tensor(wvw[:, tsl], sq3t[:, :, 0:H],
                                                sq3t[:, :, H:2 * H], Alu.add)
                    if t == NPC // 4 - 1:
                        # first-half Sins fire while the second half's
                        # matmuls stream
                        nc.scalar.activation(s[:, 0:HW2], fw[:, 0:HW2],
                                             Act.Sin, bias=0.0, scale=2 * PI)
                        nc.scalar.activation(sh[:, 0:HW2], fw[:, 0:HW2],
                                             Act.Sin, bias=0.0, scale=PI)
                nc.scalar.activation(s[:, HW2:W], fw[:, HW2:W], Act.Sin,
                                     bias=0.0, scale=2 * PI)
                nc.scalar.activation(sh[:, HW2:W], fw[:, HW2:W], Act.Sin,
                                     bias=0.0, scale=PI)
                return s, sh, wvw

            def combine_transpose(ch, s, sh, E):
                """c2/xre/xim written straight into the 3 K-section layout
                xq (each section = 16 point-blocks x 128 rows), then 12
                quarter-width DMA xbar-transposes build the next layer's
                feature-major T tiles -- no PE transposes, no psum."""
                W = NPC * H
                c2 = spool.tile([128, W], f16, tag=f"v1{ch}")
                nc.gpsimd.tensor_tensor(c2[:], sh[:], sh[:], Alu.mult)
                # xq aliases the sqw buffer (dead after the v-adds)
                xq = spool.tile([128, 3 * PB], f16, tag=f"sqw{ch}")
                q0 = xq[:, 0:PB].rearrange("p (n r) -> p n r", r=128)
                q1 = xq[:, PB:2 * PB].rearrange("p (n r) -> p n r", r=128)
                q2 = xq[:, 2 * PB:3 * PB].rearrange("p (n r) -> p n r", r=128)
                c3 = c2[:].rearrange("p (n g) -> p n g", g=H)
                E3 = E[:].rearrange("p (n g) -> p n g", g=H)
                s3 = s[:].rearrange("p (n g) -> p n g", g=H)
                # xre rows 0:128 -> sec0; rows 128:181 -> sec1[0:53]
                nc.vector.scalar_tensor_tensor(
                    q0[:, :, :], c3[:, :, 0:128], 0.5, E3[:, :, 0:128],
                    Alu.subtract, Alu.mult)
                nc.vector.scalar_tensor_tensor(
                    q1[:, :, 0:53], c3[:, :, 128:H], 0.5, E3[:, :, 128:H],
                    Alu.subtract, Alu.mult)
                # xim rows 0:75 -> sec1[53:128]; rows 75:181 -> sec2[0:106]
                nc.gpsimd.tensor_tensor(q1[:, :, 53:128], E3[:, :, 0:75],
                                        s3[:, :, 0:75], Alu.mult)
                nc.gpsimd.tensor_tensor(q2[:, :, 0:106], E3[:, :, 75:H],
                                        s3[:, :, 75:H], Alu.mult)
                # ones row (106) + harmless padding rows 107..127
                nc.vector.memset(q2[:, :, 106:128], 1.0)
                T0 = xpool.tile([128, PB], f16, tag=f"T0{ch}", name=f"T0{ch}")
                T1 = xpool.tile([128, PB], f16, tag=f"T1{ch}", name=f"T1{ch}")
                T2 = xpool.tile([128, PB], f16, tag=f"T2{ch}", name=f"T2{ch}")
                for part in range(4):
                    psl = slice(part * 512, (part + 1) * 512)
                    for sec, Tt in enumerate((T0, T1, T2)):
                        t3 = Tt[:, psl].rearrange("f (b r) -> f b r", r=128)
                        nc.sync.dma_start_transpose(
                            t3, xq[:, sec * PB + part * 512:
                                   sec * PB + (part + 1) * 512])
                return T0, T1, T2

            # ---- main loop -------------------------------------------
            def load_xa(nb):
                d = {}
                for ch in range(CPC):
                    t = apool.tile([3, PB], f16, tag=f"xa{ch}")
                    nc.sync.dma_start(out=t[:],
                                      in_=xa_d[ch, :, nb * PB:(nb + 1) * PB])
                    d[ch] = t
                return d

            X = {ch: None for ch in range(CPC)}

            def late(ch, mid):
                s, sh, wv = mid
                E = expph(ch, wv)
                X[ch] = combine_transpose(ch, s, sh, E)

            def final(ch, nb):
                T0, T1, T2 = X[ch]
                psf = pp.tile([OUT, PB], f32, tag="wav")
                for ni in range(PB // 512):
                    sl = slice(ni * 512, (ni + 1) * 512)
                    for ki, xt in enumerate((T0, T1, T2)):
                        kr = KC[ki][1] - KC[ki][0]
                        nc.tensor.matmul(psf[:, sl], lhsT=wfts[ch][ki][:],
                                         rhs=xt[0:kr, sl],
                                         start=(ki == 0), stop=(ki == 2))
                ob = spool.tile([OUT, PB], f16, tag=f"ob{ch}")
                nc.vector.tensor_copy(ob[:], psf[:])
                nc.sync.dma_start(out=out_d[ch, :, nb * PB:(nb + 1) * PB],
                                  in_=ob[:])

            # phase order: early both channels, then late both
            xan = load_xa(0)
            for nb in range(NBATCH):
                if nb == 0:
                    mid = {}
                    mid[0] = mm_early(0, 0, xan[0], X[0])
                    mid[1] = mm_early(1, 0, xan[1], X[1])
                xan_next = load_xa(nb + 1) if nb + 1 < NBATCH else None
                for li in (0, 1, 2):
                    for ch in range(CPC):
                        late(ch, mid[ch])
                        if li < 2:
                            mid[ch] = mm_early(ch, li + 1, xan[ch], X[ch])
                        else:
                            final(ch, nb)
                            if xan_next is not None:
                                mid[ch] = mm_early(ch, 0, xan_next[ch],
                                                   X[ch])
                xan = xan_next
    nc.finalize()
    return nc


def _get_graph():
    global _GRAPH
    if _GRAPH is None:
        _GRAPH = _build_graph()
    return _GRAPH


def _pack_inputs(inp, indices, model_idx, bias_idx, W0a, b0a, W0b, b0b,
                 W1a, b1a, W1b, b1b, W2a, b2a, W2b, b2b, Wf, bf):
    """Host-side gather + weight packing. Returns in_maps for 8 cores.

    Weight rows are input features in T-tile order [xre'(181); xim'(181);
    ones]; columns are the 4 group outputs [g0|g1|g2|g3].  alpha/beta are
    the affine factors mapping stored activations to true ones:
    x.re = alpha*xre', x.im = beta*xim'.
    """
    cplx = lambda a: a[..., 0] + 1j * a[..., 1]
    idn = np.eye(128, dtype=np.float16)

    def pack_hidden(Wa, Wb, ba, bb, alpha, beta):
        # columns [g0 | g1 | g2 | g3 | g4] with g4 = S0*g0 (so the
        # sum-of-squares pass covers (S*la.re)^2 too, no separate u0)
        re_rows = np.concatenate([
            R2 * alpha * Wa.real, SCALE * alpha * Wa.imag,
            SCALE * alpha * Wb.real, SCALE * alpha * Wb.imag,
            SCALE * alpha * Wa.real], axis=1)
        im_rows = np.concatenate([
            -R2 * beta * Wa.imag, SCALE * beta * Wa.real,
            -SCALE * beta * Wb.imag, SCALE * beta * Wb.real,
            -SCALE * beta * Wa.imag], axis=1)
        ones_row = np.concatenate([
            R2 * ba.real, SCALE * ba.imag + OMEGA / (2 * SCALE),
            SCALE * bb.real, SCALE * bb.imag, SCALE * ba.real])[None, :]
        return np.concatenate([re_rows, im_rows, ones_row],
                              axis=0).astype(np.float16)

    a0, b0c = -2.0, 1.0                          # L0 -> L1 factors
    a1, b1c = -2.0 * np.exp(EBIAS), np.exp(EBIAS)  # L1 -> L2, L2 -> final

    in_maps = []
    for core in range(NCORES):
        m = {k: [] for k in ("xa", "w0", "w1", "w2", "wf")}
        for j in range(CPC):
            c = core * CPC + j
            mi, bi = int(model_idx[c]), int(bias_idx[c])
            x = inp[int(indices[c])]          # [N, NIN]
            m["xa"].append(np.concatenate(
                [x.T, np.ones((1, N), np.float32)], 0).astype(np.float16))
            w0blk = np.concatenate([
                np.concatenate([R2 * W0a[mi], SCALE * W0b[mi],
                                SCALE * W0a[mi]], axis=1),
                np.concatenate([R2 * b0a[bi], SCALE * b0b[bi],
                                SCALE * b0a[bi]])[None, :],
            ], axis=0)
            m["w0"].append(w0blk.astype(np.float16))
            m["w1"].append(pack_hidden(cplx(W1a[mi]), cplx(W1b[mi]),
                                       cplx(b1a[bi]), cplx(b1b[bi]), a0, b0c))
            m["w2"].append(pack_hidden(cplx(W2a[mi]), cplx(W2b[mi]),
                                       cplx(b2a[bi]), cplx(b2b[bi]), a1, b1c))
            Wfc, bfc = cplx(Wf[mi]), cplx(bf[bi])
            wfblk = np.concatenate([
                a1 * Wfc.real, -b1c * Wfc.imag, bfc.real[None, :]],
                axis=0).astype(np.float16)
            m["wf"].append(wfblk)
        packed = {k: np.stack(v) for k, v in m.items()}
        packed["idn"] = idn
        in_maps.append(packed)
    return in_maps


def kernel(**inputs):
    inp = np.asarray(inputs["inp"], np.float32)
    args = {k: np.asarray(v) for k, v in inputs.items()}
    in_maps = _pack_inputs(
        inp, args["indices"], args["model_idx"], args["bias_idx"],
        *[np.asarray(args[k], np.float32) for k in
          ("W0a", "b0a", "W0b", "b0b", "W1a", "b1a", "W1b", "b1b",
           "W2a", "b2a", "W2b", "b2b", "Wf", "bf")])
    from concourse.bass_utils import run_bass_kernel_spmd
    nc = _get_graph()
    res = run_bass_kernel_spmd(nc, in_maps, core_ids=list(range(NCORES)))
    out = np.empty((1, C, N, OUT), np.float32)
    for core in range(NCORES):
        o = np.asarray(res.results[core]["out"])   # [CPC, OUT, N] fp16
        for j in range(CPC):
            out[0, core * CPC + j] = o[j].T.astype(np.float32)
    return out


if __name__ == "__main__":
    import jax
    import reference
    cpu = jax.devices("cpu")[0]
    with jax.default_device(cpu):
        ins = {k: np.asarray(v) for k, v in reference.setup_inputs().items()}
        exp = np.asarray(reference.reference(
            **{k: jax.device_put(v, cpu) for k, v in ins.items()}))
    got = kernel(**ins)
    rel = np.linalg.norm(got - exp) / np.linalg.norm(exp)
    print("Relative error:", rel)
